# revision 1
# baseline (speedup 1.0000x reference)
"""LIF spiking-neuron kernel for Trainium2 (Bass/Tile), 8-core SPMD.

Problem: x [T*B, F] = [8*128, 32768] f32. Per element, a scan over T=8:
    mem = mem + x_t; spike_t = (mem >= 1); mem = mem * (1 - spike_t)
Returns spikes [T*B, F] f32 (values are exactly 0.0 / 1.0).

Sharding: the F (feature) axis is split across the 8 cores (4096 cols each);
the scan over T is elementwise-independent so no communication is needed.
On each core, B=128 maps onto the SBUF partitions and T is a sequential
8-step chain per column chunk.

The tracked state is the PRE-reset membrane s_t = mem_{t-1,post-reset} + x_t:
    s_0 = x_0
    r_t = (s_t < 1) * s_t            one scalar_tensor_tensor op (DVE)
    s_{t+1} = r_t + x_{t+1}          tensor_add, split DVE / GpSimd
    spike_t = Sign(s_t - 1)          scalar engine (ACT) -> fp8 {-1, 0, +1}
The host maps the fp8 byte's sign bit to 0/1 (sign==0 happens only when
s_t == 1 exactly, where the reference's >= comparison gives 1 — the sign
bit is clear there too). This recurrence is bit-exact vs the f32 reference
(verified 0/33.5M element mismatches on hardware).

Engine budget per core (w=1024 chunks, 4 chains): the reset runs on DVE,
the add is split ~50/50 between DVE and GpSimd (1x-mode DVE ops use only
DVE's dedicated SBUF ports, so GpSimd runs fully concurrently), the spike
runs on ACT, x loads issue on the SP HWDGE ring and spike stores on the
ACT HWDGE ring so the two DMA streams don't share one FIFO. Spikes are
stored as 1 byte/element, cutting store traffic 4x: HBM traffic per core
is 16 MiB in + 4 MiB out, ~53 us at the modeled DMA rate; CoreSim
predicts ~59 us end to end (memory-bound, as targeted).
"""

import os

import numpy as np

T, B, F = 8, 128, 32768
NCORES = 8
FS = F // NCORES  # columns per core
W = 1024  # column chunk width (FS % W == 0)

# Iteration knobs (defaults are the fast path). Note: custom DVE ops
# ("fused") do not compile with this walrus build — "std" is the default.
SPIKE_ENGINE = os.environ.get("LIF_SPIKE", "act_sign")  # "act_sign" | "dve_isge"
STEP_MODE = os.environ.get("LIF_STEP", "std")  # "fused" | "std" | "dma_accum"
OUT_FMT = os.environ.get("LIF_OUT", "u8")  # "u8" | "f32"
W = int(os.environ.get("LIF_W", str(W)))
# Fraction of each add offloaded to GpSimd (its SBUF port is independent of
# DVE's dedicated ports for 1x-mode DVE ops, so they run concurrently).
GP_FRAC = float(os.environ.get("LIF_GP", "0.5"))
# Emit chunks interleaved by timestep (better engine round-robin) or not.
INTERLEAVE = os.environ.get("LIF_ILV", "0") == "1"
XBUFS = int(os.environ.get("LIF_XBUFS", "6"))
# Issue spike stores on the second HWDGE ring (ACT engine) so they don't
# queue behind loads in the SP ring's FIFO.
STORE_ENG = os.environ.get("LIF_STORE", "scalar")  # "sync" | "scalar"
# Alternate x loads across the two HWDGE rings as well.
LOAD_SPLIT = os.environ.get("LIF_LOADSPLIT", "0") == "1"
# Wide I/O: one full-width load and store per timestep (compute still
# chunked); halves the DMA op count and improves transfer efficiency.
WIDE = os.environ.get("LIF_WIDE", "0") == "1"
SBUFS = int(os.environ.get("LIF_SBUFS", "3"))
RBUFS = int(os.environ.get("LIF_RBUFS", "2"))

_cache: dict = {}

_LIF_OP_NAME = "LIF_CARRY_ANT"


def _register_lif_carry():
    """Register the fused LIF step as a custom DVE op (runtime extension of
    concourse.dve_ops): out = select(in0 < s0, in0, 0) + in1."""
    import concourse.dve_ops as dve_ops
    from concourse.dve_spec import C0, Spec, Src0, Src1, Zero, _has_src1, lower, select
    from concourse.dve_uop import DveOpSpec

    for op in dve_ops.OPS:
        if op.name == _LIF_OP_NAME:
            return op

    spec = Spec(
        body=select(Src0 < C0, Src0, Zero) + Src1,
        reference=lambda in0, in1, s0, s1, imm2: (
            np.where(in0 < s0, in0, np.float32(0.0)).astype(np.float32)
            + in1.astype(np.float32)
        ).astype(np.float32),
    )
    row = dve_ops._CUSTOM_DVE_ROW_BASE + len(dve_ops.OPS)
    assert row < 0x20, "custom-DVE row budget exhausted"
    # Self-pin the uops sha (the pin exists to catch lowering drift between
    # repo versions; for a runtime-registered op we pin what we just lowered).
    shas = {}
    for ver in ("v3", "v4"):
        try:
            s = DveOpSpec(
                name=_LIF_OP_NAME,
                opcode=row,
                uops=lower(spec, ver=ver),
                rd1_en=_has_src1(spec),
            )
            shas[ver] = s.sha(ver)
        except Exception:
            pass
    op = dve_ops.DveOp(_LIF_OP_NAME, spec, subdim=False, uops_sha=shas)
    dve_ops.OPS.append(op)
    dve_ops._SUB_OPCODE_FOR_NAME[op.name] = row
    dve_ops.CUSTOM_DVE_SPECS[op.name] = spec
    return op


def build_tile_program(nc, tc, x_ap, out_ap, reps=1):
    """Emit the per-core LIF program. x_ap: [T*B, FS] f32 DRAM; out_ap:
    [T*B, FS] DRAM (uint8 spike encoding or f32, per OUT_FMT). reps>1
    repeats the whole pass (benchmark variant for differential timing)."""
    import concourse.mybir as mybir

    dt = mybir.dt
    Alu = mybir.AluOpType
    fs = x_ap.shape[1]
    w = min(W, fs)
    assert fs % w == 0, (fs, w)
    x3 = x_ap.rearrange("(t b) f -> t b f", b=B)
    o3 = out_ap.rearrange("(t b) f -> t b f", b=B)

    lif_op = _register_lif_carry() if STEP_MODE == "fused" else None
    if OUT_FMT == "u8":
        spk_dt = dt.float8e4 if SPIKE_ENGINE == "act_sign" else dt.uint8
    else:
        spk_dt = dt.float32

    n_spk = (fs // w) * T
    have_const = (dt.float32, -1.0) in nc.const_aps.aps
    with (
        tc.tile_pool(name="cp", bufs=1) as cp,
        tc.tile_pool(name="xp", bufs=XBUFS) as xp,
        tc.tile_pool(name="sp", bufs=SBUFS) as sp,
        tc.tile_pool(name="rp", bufs=RBUFS) as rp,
        # One slot per spike tile: no slot reuse, so the ACT Sign never
        # needs a DMA-store slot-free wait (fewer waits = less event-sem
        # legalization in Bacc.compile).
        tc.tile_pool(name="op", bufs=n_spk) as op_pool,
    ):
        nbias = None
        if SPIKE_ENGINE == "act_sign" and not have_const:
            # Fallback when the -1.0 const AP isn't preregistered (e.g. the
            # run_kernel sim harness): init on ACT via Copy = in*0 + (-1).
            nbias = cp.tile([B, 1], dt.float32, tag="nbias")
            nc.scalar.activation(
                out=nbias[:],
                in_=nbias[:],
                func=mybir.ActivationFunctionType.Copy,
                bias=-1.0,
                scale=0.0,
            )
        def one_pass():
            nchunks = fs // w
            if WIDE:
                one_pass_wide(nchunks)
            elif INTERLEAVE:
                s_prev = [None] * nchunks
                for t in range(T):
                    for c in range(nchunks):
                        s_prev[c] = emit_step(c, t, s_prev[c])
            else:
                for c in range(nchunks):
                    sp_c = None
                    for t in range(T):
                        sp_c = emit_step(c, t, sp_c)

        def one_pass_wide(nchunks):
            store_dma = nc.scalar if STORE_ENG == "scalar" else nc.sync
            gp_cols = int(round(w * GP_FRAC / 256.0)) * 256
            s_prev = [None] * nchunks
            for t in range(T):
                xt = xp.tile([B, fs], dt.float32, tag="xt")
                nc.sync.dma_start(out=xt[:], in_=x3[t])
                spk = op_pool.tile([B, fs], spk_dt, tag="spk")
                for c in range(nchunks):
                    cols = slice(c * w, (c + 1) * w)
                    xc = xt[:, cols]
                    if t == 0:
                        s = xc
                    else:
                        r = rp.tile([B, w], dt.float32, tag="r")
                        nc.vector.scalar_tensor_tensor(
                            out=r[:],
                            in0=s_prev[c],
                            scalar=1.0,
                            in1=s_prev[c],
                            op0=Alu.is_lt,
                            op1=Alu.mult,
                        )
                        s = sp.tile([B, w], dt.float32, tag="s")
                        if gp_cols > 0:
                            dv = w - gp_cols
                            nc.vector.tensor_add(
                                out=s[:, :dv], in0=r[:, :dv], in1=xc[:, :dv]
                            )
                            nc.gpsimd.tensor_add(
                                out=s[:, dv:], in0=r[:, dv:], in1=xc[:, dv:]
                            )
                        else:
                            nc.vector.tensor_add(out=s[:], in0=r[:], in1=xc)
                        s = s[:]
                    if SPIKE_ENGINE == "act_sign":
                        nc.scalar.activation(
                            out=spk[:, cols],
                            in_=s,
                            func=mybir.ActivationFunctionType.Sign,
                            bias=-1.0 if have_const else nbias[:],
                        )
                    else:
                        nc.vector.tensor_scalar(
                            out=spk[:, cols],
                            in0=s,
                            scalar1=1.0,
                            scalar2=None,
                            op0=Alu.is_ge,
                        )
                    s_prev[c] = s
                store_src = (
                    spk[:].bitcast(dt.uint8) if spk_dt == dt.float8e4 else spk[:]
                )
                store_dma.dma_start(out=o3[t], in_=store_src)

        def emit_step(c, t, s_prev):
            cols = slice(c * w, (c + 1) * w)
            gp_cols = int(round(w * GP_FRAC / 256.0)) * 256
            if STEP_MODE == "dma_accum":
                # s_t = reset(s_{t-1}) + x_t with the add done by the
                # SDMA inline adder during the x load (SWDGE accum).
                s = sp.tile([B, w], dt.float32, tag="s")
                if t == 0:
                    nc.sync.dma_start(out=s[:], in_=x3[t, :, cols])
                else:
                    nc.vector.scalar_tensor_tensor(
                        out=s[:],
                        in0=s_prev[:],
                        scalar=1.0,
                        in1=s_prev[:],
                        op0=Alu.is_lt,
                        op1=Alu.mult,
                    )
                    nc.gpsimd.dma_start(
                        out=s[:], in_=x3[t, :, cols], accum_op=Alu.add
                    )
            elif t == 0:
                xt = xp.tile([B, w], dt.float32, tag="xt")
                load_dma = nc.scalar if (LOAD_SPLIT and t % 2) else nc.sync
                load_dma.dma_start(out=xt[:], in_=x3[t, :, cols])
                s = xt
            else:
                xt = xp.tile([B, w], dt.float32, tag="xt")
                load_dma = nc.scalar if (LOAD_SPLIT and t % 2) else nc.sync
                load_dma.dma_start(out=xt[:], in_=x3[t, :, cols])
                s = sp.tile([B, w], dt.float32, tag="s")
                if lif_op is not None:
                    nc.vector._custom_dve(
                        lif_op, out=s[:], in0=s_prev[:], in1=xt[:], s0=1.0
                    )
                else:
                    r = rp.tile([B, w], dt.float32, tag="r")
                    nc.vector.scalar_tensor_tensor(
                        out=r[:],
                        in0=s_prev[:],
                        scalar=1.0,
                        in1=s_prev[:],
                        op0=Alu.is_lt,
                        op1=Alu.mult,
                    )
                    if gp_cols > 0:
                        # Split the add: GpSimd's SBUF port is independent
                        # of DVE's dedicated ports (1x-mode DVE ops never
                        # use the shared pair), so these run concurrently.
                        dv = w - gp_cols
                        nc.vector.tensor_add(
                            out=s[:, :dv], in0=r[:, :dv], in1=xt[:, :dv]
                        )
                        nc.gpsimd.tensor_add(
                            out=s[:, dv:], in0=r[:, dv:], in1=xt[:, dv:]
                        )
                    else:
                        nc.vector.tensor_add(out=s[:], in0=r[:], in1=xt[:])
            spk = op_pool.tile([B, w], spk_dt, tag="spk")
            if SPIKE_ENGINE == "act_sign":
                # bias -1.0: the const AP registered in _build_nc's
                # preamble carries no Tile-tracked dep (fewer waits on
                # the Sign instruction).
                nc.scalar.activation(
                    out=spk[:],
                    in_=s[:],
                    func=mybir.ActivationFunctionType.Sign,
                    bias=-1.0 if have_const else nbias[:],
                )
            else:
                nc.vector.tensor_scalar(
                    out=spk[:],
                    in0=s[:],
                    scalar1=1.0,
                    scalar2=None,
                    op0=Alu.is_ge,
                )
            store_src = (
                spk[:].bitcast(dt.uint8)
                if spk_dt == dt.float8e4
                else spk[:]
            )
            store_dma = nc.scalar if STORE_ENG == "scalar" else nc.sync
            store_dma.dma_start(out=o3[t, :, cols], in_=store_src)
            return s

        # reps>1 is the benchmark variant (same I/O each pass), statically
        # unrolled — a tc.For_i back-edge at high trip counts wedged the NRT.
        for _ in range(reps):
            one_pass()


def _build_nc(reps=1):
    import concourse.bacc as bacc
    import concourse.mybir as mybir
    from concourse.tile import TileContext

    dt = mybir.dt
    # Bacc (not raw Bass): its compile() pass legalizes multi-wait
    # instructions via event semaphores — walrus codegen allows only one
    # sync wait per compute instruction.
    nc = bacc.Bacc(trn_type="TRN2")
    if SPIKE_ENGINE == "act_sign":
        # Register -1.0 as a const AP in the preamble (mirrors the built-in
        # 0.0/1.0 const APs) so Sign's bias read carries no Tile-tracked dep.
        t = nc.alloc_sbuf_tensor("const-float32--1.0", [128, 1], dt.float32)
        nc.gpsimd.memset(t.ap(), -1.0)
        nc.const_aps.aps[(dt.float32, -1.0)] = t.ap()
        nc.all_engine_barrier()
    x = nc.dram_tensor("x", (T * B, FS), dt.float32, kind="ExternalInput")
    out_dt = dt.uint8 if OUT_FMT == "u8" else dt.float32
    out = nc.dram_tensor("out", (T * B, FS), out_dt, kind="ExternalOutput")
    with TileContext(nc) as tc:
        build_tile_program(nc, tc, x[:], out[:], reps=reps)
    nc.compile()
    return nc


def _decode_spikes(raw: np.ndarray) -> np.ndarray:
    """Map the device output to f32 spikes (0.0/1.0)."""
    if OUT_FMT == "u8":
        if SPIKE_ENGINE == "act_sign":
            # fp8e4m3 bytes of Sign(s-1): 0xB8 (-1) -> 0, 0x00 (0) / 0x38 (+1) -> 1
            return ((raw & 0x80) == 0).astype(np.float32)
        return raw.astype(np.float32)
    if SPIKE_ENGINE == "act_sign":
        return (raw >= 0).astype(np.float32)
    return raw.astype(np.float32)


def kernel(**inputs) -> np.ndarray:
    x = np.ascontiguousarray(np.asarray(inputs["x"], dtype=np.float32))
    assert x.shape == (T * B, F), x.shape

    if "nc" not in _cache:
        _cache["nc"] = _build_nc()
    nc = _cache["nc"]

    # NTFF tracing needs antenv.axon_hooks, which this client lacks; make
    # sure a stray BASS_TRACE in the environment cannot crash the run.
    os.environ.setdefault("BASS_NEVER_TRACE", "1")

    from concourse.bass_utils import run_bass_kernel_spmd

    shards = [np.ascontiguousarray(x[:, i * FS : (i + 1) * FS]) for i in range(NCORES)]
    in_maps = [{"x": s} for s in shards]
    res = run_bass_kernel_spmd(nc, in_maps, core_ids=list(range(NCORES)))
    _cache["last_results"] = res

    outs = [np.asarray(r["out"]) for r in res.results]
    if OUT_FMT == "u8":
        outs = [o.view(np.uint8) for o in outs]
    raw = np.concatenate(outs, axis=1)
    return _decode_spikes(raw)

